# Initial kernel scaffold
#
"""MultiHeadSelfAttention3D Trainium2 kernel (8 NeuronCores, query-parallel).

Reference computation (B=1, C=64, D=H=W=16 -> N=4096, 8 heads x head_dim 8):
    qkv = w_qkv @ x_flat                  # [192, N]
    per head: S = (q^T k) / sqrt(8); P = softmax(S, axis=-1)
    out_h = v @ P^T                       # [8, N]
    out = w_proj @ concat(out_h) + b_proj
    y = gamma * out + x_flat

Sharding: each core owns 512 query positions (all 8 heads). Every core
computes K/V for all 4096 keys (cheap projections), so no collectives are
needed; per-core outputs are disjoint column slices of the result.

Layouts (per core):
  - "spread" layout: 4 heads per 128-partition tensor, head strip at
    partition 32*b (rows 32b..32b+8 live, rest zero). Two halves cover 8
    heads. This keeps every SBUF compute operand 32-partition aligned
    (hardware requirement).
  - S^T tiles [m=128 keys, n=512 queries] so exp output feeds AV matmuls
    directly (contraction over m = partitions), no transposes anywhere.
  - Two S^T tiles share one 2-bank PSUM group so each ACT exp instruction
    covers 1024 elements/partition (amortizes the ~220-cycle ACT init).
  - x carries an appended ones row, and V^T an extra ones column per head:
    the AV matmul then emits the softmax denominator as row 0 of each
    strip's output for free (no separate reduction).
  - Normalization per strip: reciprocal of the denominator row, expanded
    across the strip's 9 rows by a rank-1 matmul with e9 = [0,1,...,1],
    then one elementwise multiply into the spread attn tile.
  - QK/AV matmuls run on float32r operands (1 col/cycle vs 4 for fp32;
    measured accuracy on HW is ~1e-6 relative, i.e. near-fp32); producers
    (DVE copies, ACT exp) emit rounded float32r as the BIR verifier
    requires. Projections / normalization stay full fp32.
  - Head strips are processed in pairs with QK/exp/AV interleaved across
    the two strips, so LDWEIGHTS alternates PE row-groups and pulls ahead
    of the other strip's in-flight matmul.

Measured (bench_pair.py, same-session R=9 vs R=1 differencing; axon
per-call dispatch is 70-100 ms and drifts between sessions, so only
same-session differences are meaningful):
  ~178 us on silicon (cleanest session; noisier sessions center 172-238 us);
  rel err vs the jax fp32 reference: 2.6e-06.
  (Typing the projection inputs float32r was tried and reverted: no
  measurable speed gain, rel err degraded to 2e-04.)
Cost-model timeline for this program: 171 us. Engine budget per core:
ACT exp ~133 us busy (hard floor 109 us: 16.8M softmax elements / 128
lanes / 1.2 GHz — unreducible by any 8-core sharding), PE ~157 us busy,
DVE ~60 us, all overlapped.
"""

import numpy as np
from contextlib import ExitStack

import concourse.bass as bass
from concourse import bacc
import concourse.tile as tile
import concourse.mybir as mybir
from concourse.bass_utils import run_bass_kernel_spmd

f32 = mybir.dt.float32
f32r = mybir.dt.float32r
AF = mybir.ActivationFunctionType
ALU = mybir.AluOpType

NCORES = 8
C = 64
N = 4096
NH = 8
HD = 8
NQ = N // NCORES          # 512 queries per core
NT = N // 128             # 32 key tiles
NG = NT // 2              # 16 key-tile pairs (one exp per pair)
SCALE = float(HD) ** -0.5


def _build_nc(reps=1):
    """reps>1 chains the compute body serially (each rep's queries = scaled
    previous output) — used only for wall-clock benchmarking, where kernel
    time = (t(R) - t(1)) / (R - 1) cancels per-dispatch overhead."""
    nc = bacc.Bacc()

    xf_d = nc.declare_dram_parameter("xf", [C + 1, N], f32, isOutput=False)
    xq_d = nc.declare_dram_parameter("xq", [C, NQ], f32, isOutput=False)
    wq0_d = nc.declare_dram_parameter("wq0", [C, 128], f32, isOutput=False)
    wq1_d = nc.declare_dram_parameter("wq1", [C, 128], f32, isOutput=False)
    wk0_d = nc.declare_dram_parameter("wk0", [C, 128], f32, isOutput=False)
    wk1_d = nc.declare_dram_parameter("wk1", [C, 128], f32, isOutput=False)
    wv9_d = nc.declare_dram_parameter("wv9", [C + 1, NH * 9], f32, isOutput=False)
    wp0_d = nc.declare_dram_parameter("wp0", [128, C], f32, isOutput=False)
    wp1_d = nc.declare_dram_parameter("wp1", [128, C], f32, isOutput=False)
    e9_d = nc.declare_dram_parameter("e9", [1, 9], f32, isOutput=False)
    bq_d = nc.declare_dram_parameter("bq", [C, 1], f32, isOutput=False)
    out_d = nc.declare_dram_parameter("out", [C, NQ], f32, isOutput=True)

    with tile.TileContext(nc) as tc, ExitStack() as ctx:
        const = ctx.enter_context(tc.tile_pool(name="const", bufs=1))
        pt_pool = ctx.enter_context(tc.tile_pool(name="pt", bufs=4))
        s_ps = ctx.enter_context(tc.tile_pool(name="s_ps", bufs=2, space="PSUM"))
        o_ps_pool = ctx.enter_context(tc.tile_pool(name="o_ps", bufs=2, space="PSUM"))
        misc_ps = ctx.enter_context(tc.tile_pool(name="m_ps", bufs=2, space="PSUM"))

        # ---- load inputs ----
        # Matmuls consume DMA'd tiles directly; Bacc's wait-splitting pass
        # legalizes multi-semaphore waits (one sync-wait per lowered
        # instruction) by inserting EventSemaphore hops.
        xq_s = const.tile([C, NQ], f32, tag="xq")
        nc.sync.dma_start(xq_s[:], xq_d[:])
        w_c, w_dram = {}, {
            "wq0": wq0_d, "wq1": wq1_d, "wk0": wk0_d, "wk1": wk1_d,
            "wv9": wv9_d, "wp0": wp0_d, "wp1": wp1_d, "e9": e9_d,
        }
        for nm, d in w_dram.items():
            t = const.tile(list(d.shape), d.dtype, tag=nm)
            nc.sync.dma_start(t[:], d[:])
            w_c[nm] = t
        bq_s = const.tile([C, 1], f32, tag="bq")
        nc.sync.dma_start(bq_s[:], bq_d[:])

        # xf chunked so projection matmuls start as chunks land
        xf_s = const.tile([C + 1, N], f32, tag="xf")
        for j in range(8):
            nc.sync.dma_start(xf_s[:, j * NQ:(j + 1) * NQ],
                              xf_d[:, j * NQ:(j + 1) * NQ])
        wq_s = [w_c["wq0"], w_c["wq1"]]
        wk_s = [w_c["wk0"], w_c["wk1"]]
        wp_s = [w_c["wp0"], w_c["wp1"]]
        wv9_s = w_c["wv9"]
        e9_s = w_c["e9"]

        # (body emitted once per rep; rep>0 used only for benchmarking)
        for rep in range(reps):
            if rep > 0:
                xq_next = const.tile([C, NQ], f32, tag="xq_n")
                nc.vector.tensor_scalar_mul(xq_next[:], prev_out[:], 0.25)
                xq_s = xq_next

            # ---- projections (fp32 matmuls; outputs cast to f32r) ----
            # Q spread halves [128, NQ]
            q_sp = []
            for X in range(2):
                q_psum = misc_ps.tile([128, NQ], f32, tag="misc")
                nc.tensor.matmul(q_psum[:], lhsT=wq_s[X][:], rhs=xq_s[:],
                                 start=True, stop=True)
                q_sb = const.tile([128, NQ], f32r, tag=f"q{X}")
                nc.vector.tensor_copy(q_sb[:], q_psum[:])
                q_sp.append(q_sb)

            # K spread halves [128, N]
            k_sp = []
            for X in range(2):
                k_sb = const.tile([128, N], f32r, tag=f"k{X}")
                for j in range(8):
                    k_psum = misc_ps.tile([128, NQ], f32, tag="misc")
                    nc.tensor.matmul(k_psum[:], lhsT=wk_s[X][:],
                                     rhs=xf_s[0:C, j * NQ:(j + 1) * NQ],
                                     start=True, stop=True)
                    nc.vector.tensor_copy(k_sb[:, j * NQ:(j + 1) * NQ], k_psum[:])
                k_sp.append(k_sb)

            # V^T with ones columns [128, NT*72], groups of 9 per (tile, head).
            # Emitted after Q/K so the first QK isn't scheduled behind 32 V^T
            # matmuls; AVs only need vt_s after the first exp completes.
            vt_s = const.tile([128, NT * NH * 9], f32r, tag="vt")
            for t in range(NT):
                vt_psum = misc_ps.tile([128, NH * 9], f32, tag="misc")
                nc.tensor.matmul(vt_psum[:], lhsT=xf_s[:, t * 128:(t + 1) * 128],
                                 rhs=wv9_s[:], start=True, stop=True)
                nc.vector.tensor_copy(vt_s[:, t * 72:(t + 1) * 72], vt_psum[:])

            # ---- attention ----
            # Each head strip accumulates into its own [9, NQ] PSUM tile at
            # partition 0 (the ISA rejects matmul PSUM writes at nonzero
            # partition offsets); row 0 is the softmax denominator. Results are
            # scattered into the spread-layout attn tile at aligned 32b rows.
            attn_sp = []
            for X in range(2):
                a_sb = const.tile([128, NQ], f32, tag=f"attn{X}")
                nc.vector.memset(a_sb[:], 0.0)
                # Strips processed in pairs, interleaved per key-tile group:
                # QK matmuls of the two strips alternate PE row-groups, so
                # LDWEIGHTS pulls ahead of the other strip's in-flight MM.
                for bp in range(2):
                    pair = (2 * bp, 2 * bp + 1)
                    o_psum = {b: o_ps_pool.tile([9, NQ], f32, tag="o",
                                                 name=f"o_ps_{X}_{b}")
                              for b in pair}
                    prev = {b: None for b in pair}
                    for g in range(NG):
                        t0 = 2 * g
                        s_big, pt = {}, {}
                        for i in range(2):
                            for b in pair:
                                if i == 0 and b not in s_big:
                                    s_big[b] = s_ps.tile(
                                        [128, 2 * NQ], f32, tag="s",
                                        name=f"s_big_{X}_{b}")
                                nc.tensor.matmul(
                                    s_big[b][:, i * NQ:(i + 1) * NQ],
                                    lhsT=k_sp[X][32 * b:32 * b + 32,
                                                 (t0 + i) * 128:(t0 + i + 1) * 128],
                                    rhs=q_sp[X][32 * b:32 * b + 32, :],
                                    start=True, stop=True,
                                    tile_position=(32 * b, 0))
                        for b in pair:
                            pt[b] = pt_pool.tile([128, 2 * NQ], f32r, tag="pt",
                                                 name=f"pt_{X}_{b}")
                            nc.scalar.activation(pt[b][:], s_big[b][:], AF.Exp,
                                                 scale=SCALE)
                        for i in range(2):
                            for b in pair:
                                if prev[b] is None:
                                    continue
                                tp, ptp = prev[b]
                                h = 4 * X + b
                                nc.tensor.matmul(
                                    o_psum[b][:],
                                    lhsT=vt_s[:, (tp + i) * 72 + 9 * h:
                                              (tp + i) * 72 + 9 * h + 9],
                                    rhs=ptp[:, i * NQ:(i + 1) * NQ],
                                    start=(tp + i == 0), stop=False)
                        for b in pair:
                            prev[b] = (t0, pt[b])
                    for i in range(2):
                        for b in pair:
                            tp, ptp = prev[b]
                            h = 4 * X + b
                            nc.tensor.matmul(
                                o_psum[b][:],
                                lhsT=vt_s[:, (tp + i) * 72 + 9 * h:
                                          (tp + i) * 72 + 9 * h + 9],
                                rhs=ptp[:, i * NQ:(i + 1) * NQ],
                                start=False, stop=(i == 1))

                    for b in pair:
                        # normalize this strip now so its PSUM bank recycles:
                        # denominator is row 0; expand recip across the 9 rows
                        # via a rank-1 matmul with e9 = [0,1,1,...,1].
                        den_b = const.tile([1, NQ], f32, tag=f"den{X}{b}")
                        nc.vector.tensor_copy(den_b[:], o_psum[b][0:1, :])
                        denr_b = const.tile([1, NQ], f32, tag=f"denr{X}{b}")
                        nc.vector.reciprocal(denr_b[:], den_b[:])
                        rs_psum = misc_ps.tile([9, NQ], f32, tag="misc")
                        nc.tensor.matmul(rs_psum[:], lhsT=e9_s[:], rhs=denr_b[:],
                                         start=True, stop=True)
                        rs_sb = const.tile([9, NQ], f32, tag=f"rs{X}{b}")
                        nc.vector.tensor_copy(rs_sb[:], rs_psum[:])
                        nc.vector.tensor_tensor(
                            out=a_sb[32 * b:32 * b + 9, :],
                            in0=o_psum[b][:],
                            in1=rs_sb[:], op=ALU.mult)
                attn_sp.append(a_sb)

            # ---- output projection + bias + residual ----
            p_psum = misc_ps.tile([C, NQ], f32, tag="misc")
            nc.tensor.matmul(p_psum[:], lhsT=wp_s[0][:], rhs=attn_sp[0][:],
                             start=True, stop=False)
            nc.tensor.matmul(p_psum[:], lhsT=wp_s[1][:], rhs=attn_sp[1][:],
                             start=False, stop=True)
            out_s = const.tile([C, NQ], f32, tag="out")
            nc.vector.scalar_tensor_tensor(out=out_s[:], in0=p_psum[:], scalar=bq_s[:],
                                           in1=xq_s[:], op0=ALU.add, op1=ALU.add)
            prev_out = out_s

        nc.sync.dma_start(out_d[:], out_s[:])

    return nc


def _host_prep(x, w_qkv, w_proj, b_proj, gamma):
    xf = np.ascontiguousarray(np.asarray(x, dtype=np.float32).reshape(C, N))
    xf_aug = np.concatenate([xf, np.ones((1, N), np.float32)], axis=0)
    w_qkv = np.asarray(w_qkv, dtype=np.float32)
    w_proj = np.asarray(w_proj, dtype=np.float32)
    b_proj = np.asarray(b_proj, dtype=np.float32)
    w_q = w_qkv[0:C]
    w_k = w_qkv[C:2 * C]
    w_v = w_qkv[2 * C:3 * C]
    g = float(np.asarray(gamma).reshape(-1)[0])

    wq_sp = [np.zeros((C, 128), np.float32) for _ in range(2)]
    wk_sp = [np.zeros((C, 128), np.float32) for _ in range(2)]
    wp_sp = [np.zeros((128, C), np.float32) for _ in range(2)]
    for h in range(NH):
        X, b = divmod(h, 4)
        for d in range(HD):
            wq_sp[X][:, 32 * b + d] = w_q[8 * h + d, :]
            wk_sp[X][:, 32 * b + d] = w_k[8 * h + d, :]
            # AV output rows: ones/denominator at 32b, values at 32b+1+d
            wp_sp[X][32 * b + 1 + d, :] = g * w_proj[:, 8 * h + d]
    wv9 = np.zeros((C + 1, NH * 9), np.float32)
    for h in range(NH):
        wv9[C, 9 * h] = 1.0                       # ones column (denominator)
        wv9[0:C, 9 * h + 1:9 * h + 9] = w_v[8 * h:8 * h + 8, :].T
    e9 = np.zeros((1, 9), np.float32)
    e9[0, 1:9] = 1.0
    bq = (g * b_proj).reshape(C, 1).astype(np.float32)

    base = {
        "xf": xf_aug,
        "wq0": wq_sp[0], "wq1": wq_sp[1],
        "wk0": wk_sp[0], "wk1": wk_sp[1],
        "wv9": wv9,
        "wp0": wp_sp[0], "wp1": wp_sp[1],
        "e9": e9, "bq": bq,
    }
    in_maps = []
    for i in range(NCORES):
        m = dict(base)
        m["xq"] = np.ascontiguousarray(xf[:, i * NQ:(i + 1) * NQ])
        in_maps.append(m)
    return in_maps


_NC_CACHE = None


def _get_nc():
    global _NC_CACHE
    if _NC_CACHE is None:
        _NC_CACHE = _build_nc()
        _NC_CACHE.finalize()   # Bacc: run compile passes (wait splitting etc.)
    return _NC_CACHE


def kernel(x, w_qkv, w_proj, b_proj, gamma, _trace=False, _trace_kwargs=None):
    in_maps = _host_prep(x, w_qkv, w_proj, b_proj, gamma)
    nc = _get_nc()
    res = run_bass_kernel_spmd(nc, in_maps, list(range(NCORES)),
                               trace=_trace, **(_trace_kwargs or {}))
    out = np.concatenate([res.results[i]["out"] for i in range(NCORES)], axis=1)
    out = out.reshape(1, C, 16, 16, 16).astype(np.float32)
    if _trace:
        kernel._last_result = res
    return out



# revision 1
# speedup vs baseline: 1.3467x; 1.3467x over previous
"""MultiHeadSelfAttention3D Trainium2 kernel (8 NeuronCores, query-parallel).

Reference computation (B=1, C=64, D=H=W=16 -> N=4096, 8 heads x head_dim 8):
    qkv = w_qkv @ x_flat                  # [192, N]
    per head: S = (q^T k) / sqrt(8); P = softmax(S, axis=-1)
    out_h = v @ P^T                       # [8, N]
    out = w_proj @ concat(out_h) + b_proj
    y = gamma * out + x_flat

Sharding: each core owns 512 query positions (all 8 heads). Every core
computes K/V for all 4096 keys (cheap projections), so no collectives are
needed; per-core outputs are disjoint column slices of the result.

Layouts (per core):
  - "spread" layout: 4 heads per 128-partition tensor, head strip at
    partition 32*b (rows 32b..32b+8 live, rest zero). Two halves cover 8
    heads. This keeps every SBUF compute operand 32-partition aligned
    (hardware requirement).
  - S^T tiles [m=128 keys, n=512 queries] so exp output feeds AV matmuls
    directly (contraction over m = partitions), no transposes anywhere.
  - Two S^T tiles share one 2-bank PSUM group so each ACT exp instruction
    covers 1024 elements/partition (amortizes the ~220-cycle ACT init).
  - x carries an appended ones row, and V^T an extra ones column per head:
    the AV matmul then emits the softmax denominator as row 0 of each
    strip's output for free (no separate reduction).
  - Normalization per strip: reciprocal of the denominator row, expanded
    across the strip's 9 rows by a rank-1 matmul with e9 = [0,1,...,1],
    then one elementwise multiply into the spread attn tile.
  - QK/AV matmuls run on float32r operands (1 col/cycle vs 4 for fp32;
    measured accuracy on HW is ~1e-6 relative, i.e. near-fp32); producers
    (DVE copies, ACT exp) emit rounded float32r as the BIR verifier
    requires. Projections / normalization stay full fp32.
  - Head strips are processed in pairs with QK/exp/AV interleaved across
    the two strips, so LDWEIGHTS alternates PE row-groups and pulls ahead
    of the other strip's in-flight matmul.

Measured (bench_pair.py, same-session R=9 vs R=1 differencing; axon
per-call dispatch is 70-100 ms and drifts between sessions, so only
same-session differences are meaningful):
  ~178 us on silicon (cleanest session; noisier sessions center 172-238 us);
  rel err vs the jax fp32 reference: 2.6e-06.
  (Typing the projection inputs float32r was tried and reverted: no
  measurable speed gain, rel err degraded to 2e-04.)
Cost-model timeline for this program: 171 us. Engine budget per core:
ACT exp ~133 us busy (hard floor 109 us: 16.8M softmax elements / 128
lanes / 1.2 GHz — unreducible by any 8-core sharding), PE ~157 us busy,
DVE ~60 us, all overlapped.
"""

import numpy as np
from contextlib import ExitStack

import concourse.bass as bass
from concourse import bacc
import concourse.tile as tile
import concourse.mybir as mybir
from concourse.bass_utils import run_bass_kernel_spmd

f32 = mybir.dt.float32
f32r = mybir.dt.float32r
AF = mybir.ActivationFunctionType
ALU = mybir.AluOpType

NCORES = 8
C = 64
N = 4096
NH = 8
HD = 8
NQ = N // NCORES          # 512 queries per core
NT = N // 128             # 32 key tiles
NG = NT // 2              # 16 key-tile pairs (one exp per pair)
SCALE = float(HD) ** -0.5


def _build_nc(reps=1):
    """reps>1 chains the compute body serially (each rep's queries = scaled
    previous output) — used only for wall-clock benchmarking, where kernel
    time = (t(R) - t(1)) / (R - 1) cancels per-dispatch overhead."""
    nc = bacc.Bacc()

    xf_d = nc.declare_dram_parameter("xf", [C + 1, N], f32, isOutput=False)
    xq_d = nc.declare_dram_parameter("xq", [C, NQ], f32, isOutput=False)
    wq0_d = nc.declare_dram_parameter("wq0", [C, 128], f32, isOutput=False)
    wq1_d = nc.declare_dram_parameter("wq1", [C, 128], f32, isOutput=False)
    wk0_d = nc.declare_dram_parameter("wk0", [C, 128], f32, isOutput=False)
    wk1_d = nc.declare_dram_parameter("wk1", [C, 128], f32, isOutput=False)
    wv9_d = nc.declare_dram_parameter("wv9", [C + 1, NH * 9], f32, isOutput=False)
    wp0_d = nc.declare_dram_parameter("wp0", [128, C], f32, isOutput=False)
    wp1_d = nc.declare_dram_parameter("wp1", [128, C], f32, isOutput=False)
    e9_d = nc.declare_dram_parameter("e9", [1, 9], f32, isOutput=False)
    bq_d = nc.declare_dram_parameter("bq", [C, 1], f32, isOutput=False)
    out_d = nc.declare_dram_parameter("out", [C, NQ], f32, isOutput=True)

    with tile.TileContext(nc) as tc, ExitStack() as ctx:
        const = ctx.enter_context(tc.tile_pool(name="const", bufs=1))
        pt_pool = ctx.enter_context(tc.tile_pool(name="pt", bufs=4))
        s_ps = ctx.enter_context(tc.tile_pool(name="s_ps", bufs=2, space="PSUM"))
        o_ps_pool = ctx.enter_context(tc.tile_pool(name="o_ps", bufs=2, space="PSUM"))
        misc_ps = ctx.enter_context(tc.tile_pool(name="m_ps", bufs=2, space="PSUM"))

        # ---- load inputs ----
        # Matmuls consume DMA'd tiles directly; Bacc's wait-splitting pass
        # legalizes multi-semaphore waits (one sync-wait per lowered
        # instruction) by inserting EventSemaphore hops.
        xq_s = const.tile([C, NQ], f32, tag="xq")
        nc.sync.dma_start(xq_s[:], xq_d[:])
        w_c, w_dram = {}, {
            "wq0": wq0_d, "wq1": wq1_d, "wk0": wk0_d, "wk1": wk1_d,
            "wv9": wv9_d, "wp0": wp0_d, "wp1": wp1_d, "e9": e9_d,
        }
        for nm, d in w_dram.items():
            t = const.tile(list(d.shape), d.dtype, tag=nm)
            nc.sync.dma_start(t[:], d[:])
            w_c[nm] = t
        bq_s = const.tile([C, 1], f32, tag="bq")
        nc.sync.dma_start(bq_s[:], bq_d[:])

        # xf chunked so projection matmuls start as chunks land
        xf_s = const.tile([C + 1, N], f32, tag="xf")
        for j in range(8):
            nc.sync.dma_start(xf_s[:, j * NQ:(j + 1) * NQ],
                              xf_d[:, j * NQ:(j + 1) * NQ])
        wq_s = [w_c["wq0"], w_c["wq1"]]
        wk_s = [w_c["wk0"], w_c["wk1"]]
        wp_s = [w_c["wp0"], w_c["wp1"]]
        wv9_s = w_c["wv9"]
        e9_s = w_c["e9"]

        # (body emitted once per rep; rep>0 used only for benchmarking)
        for rep in range(reps):
            if rep > 0:
                xq_next = const.tile([C, NQ], f32, tag="xq_n")
                nc.vector.tensor_scalar_mul(xq_next[:], prev_out[:], 0.25)
                xq_s = xq_next

            # ---- projections (fp32 matmuls; outputs cast to f32r) ----
            # Q spread halves [128, NQ]
            q_sp = []
            for X in range(2):
                q_psum = misc_ps.tile([128, NQ], f32, tag="misc")
                nc.tensor.matmul(q_psum[:], lhsT=wq_s[X][:], rhs=xq_s[:],
                                 start=True, stop=True)
                q_sb = const.tile([128, NQ], f32r, tag=f"q{X}")
                nc.vector.tensor_copy(q_sb[:], q_psum[:])
                q_sp.append(q_sb)

            # K spread halves [128, N]
            k_sp = []
            for X in range(2):
                k_sb = const.tile([128, N], f32r, tag=f"k{X}")
                for j in range(8):
                    k_psum = misc_ps.tile([128, NQ], f32, tag="misc")
                    nc.tensor.matmul(k_psum[:], lhsT=wk_s[X][:],
                                     rhs=xf_s[0:C, j * NQ:(j + 1) * NQ],
                                     start=True, stop=True)
                    nc.vector.tensor_copy(k_sb[:, j * NQ:(j + 1) * NQ], k_psum[:])
                k_sp.append(k_sb)

            # V^T with ones columns [128, NT*72], groups of 9 per (tile, head).
            # Emitted after Q/K so the first QK isn't scheduled behind 32 V^T
            # matmuls; AVs only need vt_s after the first exp completes.
            vt_s = const.tile([128, NT * NH * 9], f32r, tag="vt")
            for t in range(NT):
                vt_psum = misc_ps.tile([128, NH * 9], f32, tag="misc")
                nc.tensor.matmul(vt_psum[:], lhsT=xf_s[:, t * 128:(t + 1) * 128],
                                 rhs=wv9_s[:], start=True, stop=True)
                nc.vector.tensor_copy(vt_s[:, t * 72:(t + 1) * 72], vt_psum[:])

            # ---- attention ----
            # Each head strip accumulates into its own [9, NQ] PSUM tile at
            # partition 0 (the ISA rejects matmul PSUM writes at nonzero
            # partition offsets); row 0 is the softmax denominator. Results are
            # scattered into the spread-layout attn tile at aligned 32b rows.
            attn_sp = []
            for X in range(2):
                a_sb = const.tile([128, NQ], f32, tag=f"attn{X}")
                nc.vector.memset(a_sb[:], 0.0)
                # Strips processed in pairs, interleaved per key-tile group:
                # QK matmuls of the two strips alternate PE row-groups, so
                # LDWEIGHTS pulls ahead of the other strip's in-flight MM.
                for bp in range(2):
                    pair = (2 * bp, 2 * bp + 1)
                    o_psum = {b: o_ps_pool.tile([9, NQ], f32, tag="o",
                                                 name=f"o_ps_{X}_{b}")
                              for b in pair}
                    prev = {b: None for b in pair}
                    for g in range(NG):
                        t0 = 2 * g
                        s_big, pt = {}, {}
                        for i in range(2):
                            for b in pair:
                                if i == 0 and b not in s_big:
                                    s_big[b] = s_ps.tile(
                                        [128, 2 * NQ], f32, tag="s",
                                        name=f"s_big_{X}_{b}")
                                nc.tensor.matmul(
                                    s_big[b][:, i * NQ:(i + 1) * NQ],
                                    lhsT=k_sp[X][32 * b:32 * b + 32,
                                                 (t0 + i) * 128:(t0 + i + 1) * 128],
                                    rhs=q_sp[X][32 * b:32 * b + 32, :],
                                    start=True, stop=True,
                                    tile_position=(32 * b, 0))
                        for b in pair:
                            pt[b] = pt_pool.tile([128, 2 * NQ], f32r, tag="pt",
                                                 name=f"pt_{X}_{b}")
                            nc.scalar.activation(pt[b][:], s_big[b][:], AF.Exp,
                                                 scale=SCALE)
                        for i in range(2):
                            for b in pair:
                                if prev[b] is None:
                                    continue
                                tp, ptp = prev[b]
                                h = 4 * X + b
                                nc.tensor.matmul(
                                    o_psum[b][:],
                                    lhsT=vt_s[:, (tp + i) * 72 + 9 * h:
                                              (tp + i) * 72 + 9 * h + 9],
                                    rhs=ptp[:, i * NQ:(i + 1) * NQ],
                                    start=(tp + i == 0), stop=False)
                        for b in pair:
                            prev[b] = (t0, pt[b])
                    for i in range(2):
                        for b in pair:
                            tp, ptp = prev[b]
                            h = 4 * X + b
                            nc.tensor.matmul(
                                o_psum[b][:],
                                lhsT=vt_s[:, (tp + i) * 72 + 9 * h:
                                          (tp + i) * 72 + 9 * h + 9],
                                rhs=ptp[:, i * NQ:(i + 1) * NQ],
                                start=False, stop=(i == 1))

                    for b in pair:
                        # normalize this strip now so its PSUM bank recycles:
                        # denominator is row 0; expand recip across the 9 rows
                        # via a rank-1 matmul with e9 = [0,1,1,...,1].
                        den_b = const.tile([1, NQ], f32, tag=f"den{X}{b}")
                        nc.vector.tensor_copy(den_b[:], o_psum[b][0:1, :])
                        denr_b = const.tile([1, NQ], f32, tag=f"denr{X}{b}")
                        nc.vector.reciprocal(denr_b[:], den_b[:])
                        rs_psum = misc_ps.tile([9, NQ], f32, tag="misc")
                        nc.tensor.matmul(rs_psum[:], lhsT=e9_s[:], rhs=denr_b[:],
                                         start=True, stop=True)
                        rs_sb = const.tile([9, NQ], f32, tag=f"rs{X}{b}")
                        nc.vector.tensor_copy(rs_sb[:], rs_psum[:])
                        nc.vector.tensor_tensor(
                            out=a_sb[32 * b:32 * b + 9, :],
                            in0=o_psum[b][:],
                            in1=rs_sb[:], op=ALU.mult)
                attn_sp.append(a_sb)

            # ---- output projection + bias + residual ----
            p_psum = misc_ps.tile([C, NQ], f32, tag="misc")
            nc.tensor.matmul(p_psum[:], lhsT=wp_s[0][:], rhs=attn_sp[0][:],
                             start=True, stop=False)
            nc.tensor.matmul(p_psum[:], lhsT=wp_s[1][:], rhs=attn_sp[1][:],
                             start=False, stop=True)
            out_s = const.tile([C, NQ], f32, tag="out")
            nc.vector.scalar_tensor_tensor(out=out_s[:], in0=p_psum[:], scalar=bq_s[:],
                                           in1=xq_s[:], op0=ALU.add, op1=ALU.add)
            prev_out = out_s

        nc.sync.dma_start(out_d[:], out_s[:])

    return nc


def _host_prep(x, w_qkv, w_proj, b_proj, gamma):
    xf = np.ascontiguousarray(np.asarray(x, dtype=np.float32).reshape(C, N))
    xf_aug = np.concatenate([xf, np.ones((1, N), np.float32)], axis=0)
    w_qkv = np.asarray(w_qkv, dtype=np.float32)
    w_proj = np.asarray(w_proj, dtype=np.float32)
    b_proj = np.asarray(b_proj, dtype=np.float32)
    w_q = w_qkv[0:C]
    w_k = w_qkv[C:2 * C]
    w_v = w_qkv[2 * C:3 * C]
    g = float(np.asarray(gamma).reshape(-1)[0])

    wq_sp = [np.zeros((C, 128), np.float32) for _ in range(2)]
    wk_sp = [np.zeros((C, 128), np.float32) for _ in range(2)]
    wp_sp = [np.zeros((128, C), np.float32) for _ in range(2)]
    for h in range(NH):
        X, b = divmod(h, 4)
        for d in range(HD):
            wq_sp[X][:, 32 * b + d] = w_q[8 * h + d, :]
            wk_sp[X][:, 32 * b + d] = w_k[8 * h + d, :]
            # AV output rows: ones/denominator at 32b, values at 32b+1+d
            wp_sp[X][32 * b + 1 + d, :] = g * w_proj[:, 8 * h + d]
    wv9 = np.zeros((C + 1, NH * 9), np.float32)
    for h in range(NH):
        wv9[C, 9 * h] = 1.0                       # ones column (denominator)
        wv9[0:C, 9 * h + 1:9 * h + 9] = w_v[8 * h:8 * h + 8, :].T
    e9 = np.zeros((1, 9), np.float32)
    e9[0, 1:9] = 1.0
    bq = (g * b_proj).reshape(C, 1).astype(np.float32)

    base = {
        "xf": xf_aug,
        "wq0": wq_sp[0], "wq1": wq_sp[1],
        "wk0": wk_sp[0], "wk1": wk_sp[1],
        "wv9": wv9,
        "wp0": wp_sp[0], "wp1": wp_sp[1],
        "e9": e9, "bq": bq,
    }
    in_maps = []
    for i in range(NCORES):
        m = dict(base)
        m["xq"] = np.ascontiguousarray(xf[:, i * NQ:(i + 1) * NQ])
        in_maps.append(m)
    return in_maps


_NC_CACHE = None


def _get_nc():
    global _NC_CACHE
    if _NC_CACHE is None:
        _NC_CACHE = _build_nc()
        _NC_CACHE.finalize()   # Bacc: run compile passes (wait splitting etc.)
    return _NC_CACHE


def kernel(x, w_qkv, w_proj, b_proj, gamma, _trace=False, _trace_kwargs=None):
    in_maps = _host_prep(x, w_qkv, w_proj, b_proj, gamma)
    nc = _get_nc()
    res = run_bass_kernel_spmd(nc, in_maps, list(range(NCORES)),
                               trace=_trace, **(_trace_kwargs or {}))
    out = np.concatenate([res.results[i]["out"] for i in range(NCORES)], axis=1)
    out = out.reshape(1, C, 16, 16, 16).astype(np.float32)
    if _trace:
        kernel._last_result = res
    return out

